# revision 1
# baseline (speedup 1.0000x reference)
"""DKVMN forward kernel for Trainium2 (8 NeuronCores, batch-parallel).

Strategy (per core, 8 batches):
  Phase A: indirect-DMA gather of key/value embedding rows (device-side
    index math), PE transposes, PE matmuls for softmax logits / erase /
    add gates, softmax via DVE+ACT.
  Scan (t = 0..199, fully unrolled): per step one PE-stationary load of a
    [9,128] block-selector (8 batches' attention weights placed in a
    (c,v) selector layout + a ones row) and 4 matmuls against per-group
    zero-padded [-e | a] moving operands produce PSUM [1 - w*e | w*a]
    directly.  DVE then needs only 2 tensor_tensor passes:
       T = Mv * A ;  Mv' = T + B.
    Reads r_t = w_t . Mv_{t-1} are 4 tiny PE matmuls with [128,2]
    stationaries, drained by ACT and DMA'd to DRAM scratch.
  Phase C: reload reads, PE transpose, two accumulated matmuls with f_W,
    tanh (+bias), p_W matvec, sigmoid -> [1, 1664] output per core.

Host side only slices/pads/reinterprets inputs and reassembles outputs.
"""

import numpy as np

B, S, DK, DV, NQ = 64, 200, 128, 64, 10000
NCORES = 8
BL = B // NCORES          # 8 batches per core
TP = 208                  # t padded to 13*128/8
NTILE = 13                # (t,b)-row tiles of 128 -> 1664 rows
ROWS = TP * BL            # 1664
CH = 10                   # scan chunk length (steps)
NCH = S // CH             # 8 chunks

_CACHE = {}


def _build(nrep=1, abl=()):
    import concourse.bacc as bacc
    import concourse.bass as bass
    import concourse.mybir as mybir
    from concourse.tile import TileContext
    from concourse.masks import make_identity

    fp32 = mybir.dt.float32
    i32 = mybir.dt.int32
    AL = mybir.AluOpType
    AF = mybir.ActivationFunctionType
    AX = mybir.AxisListType

    nc = bacc.Bacc(None)

    QID = nc.dram_tensor("qid32", [TP, 2 * BL], i32, kind="ExternalInput")
    COR = nc.dram_tensor("cor32", [TP, 2 * BL], i32, kind="ExternalInput")
    KEMB = nc.dram_tensor("key_emb", [NQ, DK], fp32, kind="ExternalInput")
    VEMB = nc.dram_tensor("value_emb", [2 * NQ, DK], fp32, kind="ExternalInput")
    MK = nc.dram_tensor("Mk", [DV, DK], fp32, kind="ExternalInput")
    MV0 = nc.dram_tensor("Mv0", [DV, DK], fp32, kind="ExternalInput")
    FW = nc.dram_tensor("f_W", [2 * DK, DK], fp32, kind="ExternalInput")
    FB = nc.dram_tensor("f_b", [DK], fp32, kind="ExternalInput")
    EW = nc.dram_tensor("e_W", [DK, DK], fp32, kind="ExternalInput")
    EB = nc.dram_tensor("e_b", [DK], fp32, kind="ExternalInput")
    AW = nc.dram_tensor("a_W", [DK, DK], fp32, kind="ExternalInput")
    AB_ = nc.dram_tensor("a_b", [DK], fp32, kind="ExternalInput")
    PW = nc.dram_tensor("p_W", [DK, 1], fp32, kind="ExternalInput")
    PB = nc.dram_tensor("p_b", [1], fp32, kind="ExternalInput")

    W_D = nc.dram_tensor("w_scratch", [ROWS, DV], fp32, kind="Internal")
    EA_D = nc.dram_tensor("ea_scratch", [ROWS, 2 * DK], fp32, kind="Internal")
    R_D = nc.dram_tensor("reads_scratch", [ROWS, DK], fp32, kind="Internal")
    OUT = nc.dram_tensor("out", [1, ROWS], fp32, kind="ExternalOutput")

    with TileContext(nc) as tc:
        with tc.tile_pool(name="const", bufs=1) as const, \
             tc.tile_pool(name="big", bufs=1) as bigp, \
             tc.tile_pool(name="scan", bufs=1) as scanp, \
             tc.tile_pool(name="work", bufs=3) as work, \
             tc.tile_pool(name="small", bufs=4) as small, \
             tc.tile_pool(name="mv", bufs=2) as mvp, \
             tc.tile_pool(name="tt", bufs=2) as ttp, \
             tc.tile_pool(name="rc", bufs=2) as rcp:

            psK_cm = tc.tile_pool(name="psK", bufs=2, space="PSUM")
            psK = psK_cm.__enter__()

            # ---------------- constants ----------------
            ident = const.tile([128, 128], fp32, tag="ident")
            make_identity(nc, ident[:])

            mk_sb = const.tile([DV, DK], fp32, tag="mk_sb")
            nc.sync.dma_start(mk_sb[:], MK[:])
            mkT_ps = psK.tile([128, 512], fp32, space="PSUM", tag="kps")
            nc.tensor.transpose(mkT_ps[0:DK, 0:DV], mk_sb[:], ident[0:DV, 0:DV])
            mkT = const.tile([DK, DV], fp32, tag="mkT")
            nc.vector.tensor_copy(mkT[:], mkT_ps[0:DK, 0:DV])

            eaW = const.tile([DK, 2 * DK], fp32, tag="eaW")
            nc.sync.dma_start(eaW[:, 0:DK], EW[:])
            nc.sync.dma_start(eaW[:, DK:2 * DK], AW[:])
            eab_row = const.tile([1, 2 * DK], fp32, tag="eab_row")
            nc.sync.dma_start(eab_row[0:1, 0:DK], EB[:].rearrange("(o k) -> o k", o=1))
            nc.sync.dma_start(eab_row[0:1, DK:2 * DK], AB_[:].rearrange("(o k) -> o k", o=1))
            ones_row = const.tile([1, DK], fp32, tag="ones_row")
            nc.vector.memset(ones_row[:], 1.0)
            onesCH = const.tile([1, CH * 128], fp32, tag="onesCH")
            nc.vector.memset(onesCH[:], 1.0)
            patCH2 = const.tile([1, CH * 512], fp32, tag="patCH2")
            nc.vector.memset(patCH2[:], 0.0)
            nc.vector.memset(
                patCH2[:].rearrange("p (t x) -> p t x", x=256)[:, :, 0:DK], 1.0)

            fW1 = const.tile([DK, DK], fp32, tag="fW1")
            nc.sync.dma_start(fW1[:], FW[0:DK, :])
            fW2 = const.tile([DK, DK], fp32, tag="fW2")
            nc.sync.dma_start(fW2[:], FW[DK:2 * DK, :])
            fb_col = const.tile([DK, 1], fp32, tag="fb_col")
            nc.sync.dma_start(fb_col[:], FB[:].rearrange("(k o) -> k o", o=1))
            pW = const.tile([DK, 1], fp32, tag="pW")
            nc.sync.dma_start(pW[:], PW[:])
            pb_t = const.tile([1, 1], fp32, tag="pb_t")
            nc.sync.dma_start(pb_t[:], PB[:].rearrange("(o k) -> o k", o=1))

            mv0_t = const.tile([128, DK], fp32, tag="mv0_t")
            nc.sync.dma_start(mv0_t[0:DV, :], MV0[:])
            nc.sync.dma_start(mv0_t[DV:128, :], MV0[:])

            psK_cm.__exit__(None, None, None)
            for rep in range(nrep):
                psA_cm = tc.tile_pool(name="psA", bufs=3, space="PSUM")
                psA = psA_cm.__enter__()
                # ---------------- indices ----------------
                qidx = const.tile([128, 16], i32, tag="qidx")
                cidx = const.tile([128, 16], i32, tag="cidx")
                vidx = const.tile([128, 16], i32, tag="vidx")
                # row r = 128c + p ; p = tl*8 + b ; t = 16c + tl
                qsrc = QID[:].rearrange("(c tl) (b two) -> tl b c two", tl=16, two=2)[:, :, :, 0]
                nc.sync.dma_start(qidx[:, 0:NTILE], qsrc)
                csrc = COR[:].rearrange("(c tl) (b two) -> tl b c two", tl=16, two=2)[:, :, :, 0]
                nc.sync.dma_start(cidx[:, 0:NTILE], csrc)
                nc.vector.scalar_tensor_tensor(
                    out=vidx[:, 0:NTILE], in0=cidx[:, 0:NTILE], scalar=NQ,
                    in1=qidx[:, 0:NTILE], op0=AL.mult, op1=AL.add)

                # ---------------- gather + transpose ----------------
                kT = bigp.tile([DK, ROWS], fp32, tag="kT")
                vT = bigp.tile([DK, ROWS], fp32, tag="vT")
                for c in range(NTILE):
                    sl = slice(128 * c, 128 * c + 128)
                    k_t = work.tile([128, DK], fp32, tag="gath")
                    nc.gpsimd.indirect_dma_start(
                        out=k_t[:], out_offset=None, in_=KEMB[:],
                        in_offset=bass.IndirectOffsetOnAxis(ap=qidx[:, c:c + 1], axis=0))
                    kt_ps = psA.tile([128, 512], fp32, space="PSUM", tag="tps")
                    nc.tensor.transpose(kt_ps[:, 0:128], k_t[:], ident[:])
                    nc.vector.tensor_copy(kT[:, sl], kt_ps[:, 0:128])

                    v_t = work.tile([128, DK], fp32, tag="gath")
                    nc.gpsimd.indirect_dma_start(
                        out=v_t[:], out_offset=None, in_=VEMB[:],
                        in_offset=bass.IndirectOffsetOnAxis(ap=vidx[:, c:c + 1], axis=0))
                    vt_ps = psA.tile([128, 512], fp32, space="PSUM", tag="tps")
                    nc.tensor.transpose(vt_ps[:, 0:128], v_t[:], ident[:])
                    nc.vector.tensor_copy(vT[:, sl], vt_ps[:, 0:128])

                # ---------------- gates: w softmax, e, a ----------------
                wT2 = bigp.tile([128, ROWS], fp32, tag="wT2")   # rows 0-63: wT ; rows 64-127: copy
                for c in range(NTILE):
                    sl = slice(128 * c, 128 * c + 128)
                    lps = psA.tile([128, 512], fp32, space="PSUM", tag="tps")
                    nc.tensor.matmul(lps[:, 0:DV], lhsT=kT[:, sl], rhs=mkT[:],
                                     start=True, stop=True)
                    negmax = small.tile([128, 1], fp32, tag="nm")
                    nc.vector.tensor_reduce(negmax[:], lps[:, 0:DV], AX.X, AL.max,
                                            negate=True)
                    exp_sb = work.tile([128, DV], fp32, tag="exp")
                    sumexp = small.tile([128, 1], fp32, tag="se")
                    nc.scalar.activation(exp_sb[:], lps[:, 0:DV], AF.Exp,
                                         bias=negmax[:, 0:1], accum_out=sumexp[:, 0:1])
                    rec = small.tile([128, 1], fp32, tag="rec")
                    nc.vector.reciprocal(rec[:], sumexp[:])
                    w_sb = work.tile([128, DV], fp32, tag="wsb")
                    nc.vector.tensor_scalar_mul(w_sb[:], exp_sb[:], rec[:, 0:1])
                    nc.sync.dma_start(W_D[128 * c:128 * c + 128, :], w_sb[:])
                    # transpose w for the read-selector
                    wt_ps = psA.tile([128, 512], fp32, space="PSUM", tag="tps")
                    nc.tensor.transpose(wt_ps[0:DV, 0:128], w_sb[:], ident[:])
                    nc.vector.tensor_copy(wT2[0:DV, sl], wt_ps[0:DV, 0:128])

                    eps_full = psA.tile([128, 512], fp32, space="PSUM", tag="tps")
                    eps = eps_full[:, 0:2 * DK]
                    nc.tensor.matmul(eps, lhsT=vT[:, sl], rhs=eaW[:],
                                     start=True, stop=False)
                    nc.tensor.matmul(eps, lhsT=ones_row[:], rhs=eab_row[:],
                                     start=False, stop=True)
                    ea_sb = work.tile([128, 2 * DK], fp32, tag="easb")
                    nc.scalar.activation(ea_sb[:, 0:DK], eps_full[:, 0:DK], AF.Sigmoid)
                    nc.scalar.activation(ea_sb[:, DK:2 * DK], eps_full[:, DK:2 * DK], AF.Tanh)
                    # negate e half (store [-e | a])
                    nc.vector.tensor_scalar_mul(ea_sb[:, 0:DK], ea_sb[:, 0:DK], -1.0)
                    nc.sync.dma_start(EA_D[128 * c:128 * c + 128, :], ea_sb[:])

                # replicate wT to partitions 64-127 (SBUF->SBUF partition shift)
                nc.sync.dma_start(wT2[DV:128, :], wT2[0:DV, :])

                # read-selector: WcvZ[(c,v), (t, g, c')] = w_t[2g+c, v] if c'==c else 0
                wcvz = scanp.tile([128, S * BL], fp32, tag="wcvz")
                nc.vector.memset(wcvz[:], 0.0)
                wv_u = wT2[0:DV, :].rearrange("p (t b) -> p t b", b=BL)
                wz_u = wcvz[0:DV, :].rearrange("p (t g c) -> p t g c", g=4, c=2)
                nc.vector.tensor_copy(wz_u[:, :, :, 0], wv_u[:, 0:S, 0::2])
                wv_l = wT2[DV:128, :].rearrange("p (t b) -> p t b", b=BL)
                wz_l = wcvz[DV:128, :].rearrange("p (t g c) -> p t g c", g=4, c=2)
                nc.vector.tensor_copy(wz_l[:, :, :, 1], wv_l[:, 0:S, 1::2])

                psA_cm.__exit__(None, None, None)
                psS_cm = tc.tile_pool(name="psS", bufs=2, space="PSUM")
                psS = psS_cm.__enter__()
                psRp_cm = tc.tile_pool(name="psRp", bufs=2, space="PSUM")
                psRp = psRp_cm.__enter__()

                # ---------------- Mv init ----------------
                mv_cur = mvp.tile([128, 4 * DK], fp32, tag="mv")
                for g in range(4):
                    nc.vector.tensor_copy(mv_cur[:, DK * g:DK * g + DK], mv0_t[:])

                # ---------------- scan buffers ----------------
                w9b = []
                ea9b = []
                for i in range(2):
                    t9 = scanp.tile([9, CH * 128], fp32, tag=f"w9_{i}")
                    nc.vector.memset(t9[0:8, :], 0.0)
                    nc.sync.dma_start(t9[8:9, :], onesCH[:])
                    w9b.append(t9)
                    gl = []
                    for j in range(2):
                        te = scanp.tile([9, CH * 512], fp32, tag=f"ea9_{i}_{j}")
                        nc.vector.memset(te[0:8, :], 0.0)
                        nc.sync.dma_start(te[8:9, :], patCH2[:])
                        gl.append(te)
                    ea9b.append(gl)

                # zero the pad region of reads scratch
                zpad = const.tile([64, DK], fp32, tag="zpad")
                nc.vector.memset(zpad[:], 0.0)
                nc.sync.dma_start(R_D[S * BL:ROWS, :], zpad[:])

                # ---------------- the scan ----------------
                rc = None
                for ch in range(NCH):
                    buf = ch % 2
                    t0 = ch * CH
                    for b in range(BL) if "refill" not in abl else []:
                        h = b % 2
                        dst_w = w9b[buf][b:b + 1, :].rearrange(
                            "p (t x) -> p t x", x=128)[:, :, 64 * h:64 * h + 64]
                        src_w = W_D[:].rearrange("(t b) v -> b t v", b=BL)[b, t0:t0 + CH, :]
                        nc.sync.dma_start(dst_w, src_w.rearrange("(o t) v -> o t v", o=1))
                        j, h2 = b // 4, (b // 2) % 2
                        dst_e = ea9b[buf][j][b:b + 1, :].rearrange(
                            "p (t x) -> p t x", x=512)[:, :, 256 * h2:256 * h2 + 256]
                        src_e = EA_D[:].rearrange("(t b) k -> b t k", b=BL)[b, t0:t0 + CH, :]
                        nc.sync.dma_start(dst_e, src_e.rearrange("(o t) k -> o t k", o=1))

                    for tl in range(CH):
                        t = t0 + tl
                        psab = psS.tile([128, 1024], fp32, space="PSUM", tag="psab")
                        if "abmm" not in abl:
                            for j in range(2):
                                nc.tensor.matmul(
                                    psab[:, 512 * j:512 * j + 512],
                                    lhsT=w9b[buf][0:9, 128 * tl:128 * tl + 128],
                                    rhs=ea9b[buf][j][0:9, 512 * tl:512 * tl + 512],
                                    start=True, stop=True)
                        psr = psRp.tile([8, 512], fp32, space="PSUM", tag="psr")
                        if "readmm" not in abl:
                            nc.tensor.matmul(
                                psr[:], lhsT=wcvz[:, 8 * t:8 * t + 8],
                                rhs=mv_cur[:], start=True, stop=True)
                        if "dve" not in abl:
                            psab_v = psab[:].rearrange("p (g x) -> p g x", g=4)
                            tT = ttp.tile([128, 4 * DK], fp32, tag="tt")
                            nc.vector.tensor_tensor(
                                out=tT[:].rearrange("p (g x) -> p g x", g=4),
                                in0=mv_cur[:].rearrange("p (g x) -> p g x", g=4),
                                in1=psab_v[:, :, 0:DK], op=AL.mult)
                            mv_next = mvp.tile([128, 4 * DK], fp32, tag="mv")
                            nc.vector.tensor_tensor(
                                out=mv_next[:].rearrange("p (g x) -> p g x", g=4),
                                in0=tT[:].rearrange("p (g x) -> p g x", g=4),
                                in1=psab_v[:, :, DK:2 * DK], op=AL.add)
                            mv_cur = mv_next
                        # reads drain
                        if "drain" in abl:
                            continue
                        if t % 4 == 0:
                            rc = rcp.tile([8, 4 * 512], fp32, tag="rc")
                        nc.scalar.copy(rc[:, 512 * (t % 4):512 * (t % 4) + 512], psr[:])
                        if t % 4 == 3 and "rcdma" not in abl:
                            for g in range(4):
                                dstg = R_D[:].rearrange(
                                    "(t b) k -> t b k", b=BL)[t - 3:t + 1, 2 * g:2 * g + 2, :]
                                srcg = rc[2 * g:2 * g + 2, :].rearrange(
                                    "c (t x) -> c t x", x=512)[:, :, 128 * g:128 * g + 128]
                                nc.sync.dma_start(
                                    dstg.rearrange("t c k -> c t k"), srcg)

                psRp_cm.__exit__(None, None, None)
                psS_cm.__exit__(None, None, None)
                psC_cm = tc.tile_pool(name="psC", bufs=3, space="PSUM")
                psC = psC_cm.__enter__()

                # ---------------- phase C ----------------
                readsT = bigp.tile([DK, ROWS], fp32, tag="readsT")
                for c in range(NTILE):
                    sl = slice(128 * c, 128 * c + 128)
                    r_t = work.tile([128, DK], fp32, tag="gath")
                    nc.sync.dma_start(r_t[:], R_D[128 * c:128 * c + 128, :])
                    rt_ps = psC.tile([128, 512], fp32, space="PSUM", tag="cps")
                    nc.tensor.transpose(rt_ps[:, 0:128], r_t[:], ident[:])
                    nc.vector.tensor_copy(readsT[:, sl], rt_ps[:, 0:128])

                fT = bigp.tile([DK, ROWS], fp32, tag="fT")
                out_sb = const.tile([1, ROWS], fp32, tag="out_sb")
                for c0 in range(0, ROWS, 512):
                    w_ = min(512, ROWS - c0)
                    sl = slice(c0, c0 + w_)
                    fps = psC.tile([128, 512], fp32, space="PSUM", tag="cps")
                    nc.tensor.matmul(fps[:, 0:w_], lhsT=fW1[:], rhs=readsT[:, sl],
                                     start=True, stop=False)
                    nc.tensor.matmul(fps[:, 0:w_], lhsT=fW2[:], rhs=kT[:, sl],
                                     start=False, stop=True)
                    nc.scalar.activation(fT[:, sl], fps[:, 0:w_], AF.Tanh,
                                         bias=fb_col[:, 0:1])
                    pps = psC.tile([2, 512], fp32, space="PSUM", tag="cpr")
                    nc.tensor.matmul(pps[0:1, 0:w_], lhsT=pW[:], rhs=fT[:, sl],
                                     start=True, stop=True)
                    nc.scalar.activation(out_sb[0:1, sl], pps[0:1, 0:w_], AF.Sigmoid,
                                         bias=pb_t[0:1, 0:1])
                nc.sync.dma_start(OUT[:], out_sb[:])
                psC_cm.__exit__(None, None, None)

    nc.finalize()
    return nc


def make_in_maps(inputs):
    def prep_idx(a):
        # [BL, S] int -> t-major padded little-endian int32 view [TP, 2*BL]
        a = np.ascontiguousarray(np.asarray(a).astype(np.int64, copy=False).T)  # [S, BL]
        v = a.view(np.int32).reshape(S, 2 * BL)
        out = np.zeros((TP, 2 * BL), np.int32)
        out[:S] = v
        return out

    common = {
        "key_emb": np.ascontiguousarray(inputs["key_emb"], np.float32),
        "value_emb": np.ascontiguousarray(inputs["value_emb"], np.float32),
        "Mk": np.ascontiguousarray(inputs["Mk"], np.float32),
        "Mv0": np.ascontiguousarray(inputs["Mv0"], np.float32),
        "f_W": np.ascontiguousarray(inputs["f_W"], np.float32),
        "f_b": np.ascontiguousarray(inputs["f_b"], np.float32),
        "e_W": np.ascontiguousarray(inputs["e_W"], np.float32),
        "e_b": np.ascontiguousarray(inputs["e_b"], np.float32),
        "a_W": np.ascontiguousarray(inputs["a_W"], np.float32),
        "a_b": np.ascontiguousarray(inputs["a_b"], np.float32),
        "p_W": np.ascontiguousarray(inputs["p_W"], np.float32),
        "p_b": np.ascontiguousarray(inputs["p_b"], np.float32),
    }
    in_maps = []
    for core in range(NCORES):
        bs = slice(core * BL, core * BL + BL)
        m = dict(common)
        m["qid32"] = prep_idx(np.asarray(inputs["question_seq"])[bs])
        m["cor32"] = prep_idx(np.asarray(inputs["correctness_seq"])[bs])
        in_maps.append(m)
    return in_maps


def kernel(**inputs):
    from concourse.bass_utils import run_bass_kernel_spmd

    if "nc" not in _CACHE:
        _CACHE["nc"] = _build()
    nc = _CACHE["nc"]
    in_maps = make_in_maps(inputs)
    _CACHE["in_maps"] = in_maps
    res = run_bass_kernel_spmd(nc, in_maps, core_ids=list(range(NCORES)))
    out = np.empty((B, S), np.float32)
    for core in range(NCORES):
        flat = res.results[core]["out"].reshape(ROWS)
        out[core * BL:(core + 1) * BL, :] = flat[:S * BL].reshape(S, BL).T
    return out



# revision 12
# speedup vs baseline: 1.8406x; 1.8406x over previous
"""DKVMN forward kernel for Trainium2 (8 NeuronCores, batch-parallel).

Strategy (per core, 8 batches):
  Phase A: indirect-DMA gather of key/value embedding rows (device-side
    index math), PE transposes, PE matmuls for softmax logits / erase /
    add gates, softmax via DVE+ACT.
  Scan (t = 0..199, fully unrolled): per step one PE-stationary load of a
    [9,128] block-selector (8 batches' attention weights placed in a
    (c,v) selector layout + a ones row) and 4 matmuls against per-group
    zero-padded [-e | a] moving operands produce PSUM [1 - w*e | w*a]
    directly.  DVE then needs only 2 tensor_tensor passes:
       T = Mv * A ;  Mv' = T + B.
    Reads r_t = w_t . Mv_{t-1} are 4 tiny PE matmuls with [128,2]
    stationaries, drained by ACT and DMA'd to DRAM scratch.
  Phase C: reload reads, PE transpose, two accumulated matmuls with f_W,
    tanh (+bias), p_W matvec, sigmoid -> [1, 1664] output per core.

Host side only slices/pads/reinterprets inputs and reassembles outputs.
"""

import numpy as np

B, S, DK, DV, NQ = 64, 200, 128, 64, 10000
NCORES = 8
BL = B // NCORES          # 8 batches per core
TP = 208                  # t padded to 13*128/8
NTILE = 13                # (t,b)-row tiles of 128 -> 1664 rows
ROWS = TP * BL            # 1664
CH = 10                   # scan chunk length (steps)
NCH = S // CH             # 8 chunks

_CACHE = {}


def _build(nrep=1, abl=()):
    import concourse.bacc as bacc
    import concourse.bass as bass
    import concourse.mybir as mybir
    from concourse.tile import TileContext
    from concourse.masks import make_identity

    fp32 = mybir.dt.float32
    f32r = mybir.dt.float32r
    i32 = mybir.dt.int32

    def R(ap):
        return ap.bitcast(f32r)
    AL = mybir.AluOpType
    AF = mybir.ActivationFunctionType
    AX = mybir.AxisListType

    nc = bacc.Bacc(None)

    QID = nc.dram_tensor("qid32", [TP, 2 * BL], i32, kind="ExternalInput")
    COR = nc.dram_tensor("cor32", [TP, 2 * BL], i32, kind="ExternalInput")
    KEMB = nc.dram_tensor("key_emb", [NQ, DK], fp32, kind="ExternalInput")
    VEMB = nc.dram_tensor("value_emb", [2 * NQ, DK], fp32, kind="ExternalInput")
    MK = nc.dram_tensor("Mk", [DV, DK], fp32, kind="ExternalInput")
    MV0 = nc.dram_tensor("Mv0", [DV, DK], f32r, kind="ExternalInput")
    FW = nc.dram_tensor("f_W", [2 * DK, DK], f32r, kind="ExternalInput")
    FB = nc.dram_tensor("f_b", [DK], fp32, kind="ExternalInput")
    EW = nc.dram_tensor("e_W", [DK, DK], f32r, kind="ExternalInput")
    EB = nc.dram_tensor("e_b", [DK], f32r, kind="ExternalInput")
    AW = nc.dram_tensor("a_W", [DK, DK], f32r, kind="ExternalInput")
    AB_ = nc.dram_tensor("a_b", [DK], f32r, kind="ExternalInput")
    PW = nc.dram_tensor("p_W", [DK, 1], f32r, kind="ExternalInput")
    PB = nc.dram_tensor("p_b", [1], fp32, kind="ExternalInput")

    W_D = nc.dram_tensor("w_scratch", [ROWS, DV], f32r, kind="Internal")
    EA_D = nc.dram_tensor("ea_scratch", [ROWS, 2 * DK], f32r, kind="Internal")
    R_D = nc.dram_tensor("reads_scratch", [ROWS, DK], fp32, kind="Internal")
    OUT = nc.dram_tensor("out", [1, ROWS], fp32, kind="ExternalOutput")

    with TileContext(nc) as tc:
        with tc.tile_pool(name="const", bufs=1) as const, \
             tc.tile_pool(name="big", bufs=1) as bigp, \
             tc.tile_pool(name="scan", bufs=1) as scanp, \
             tc.tile_pool(name="work", bufs=3) as work, \
             tc.tile_pool(name="small", bufs=4) as small, \
             tc.tile_pool(name="mv", bufs=2) as mvp, \
             tc.tile_pool(name="tt", bufs=2) as ttp, \
             tc.tile_pool(name="rc", bufs=2) as rcp:

            psK_cm = tc.tile_pool(name="psK", bufs=2, space="PSUM")
            psK = psK_cm.__enter__()

            # ---------------- constants ----------------
            ident = const.tile([128, 128], fp32, tag="ident")
            make_identity(nc, ident[:])

            mk_sb = const.tile([DV, DK], fp32, tag="mk_sb")
            nc.sync.dma_start(mk_sb[:], MK[:])
            mkT_ps = psK.tile([128, 512], fp32, space="PSUM", tag="kps")
            nc.tensor.transpose(mkT_ps[0:DK, 0:DV], mk_sb[:], ident[0:DV, 0:DV])
            mkT = const.tile([DK, DV], f32r, tag="mkT")
            nc.vector.tensor_copy(mkT[:], mkT_ps[0:DK, 0:DV])

            eaW = const.tile([DK, 2 * DK], f32r, tag="eaW")
            nc.sync.dma_start(eaW[:, 0:DK], EW[:])
            nc.sync.dma_start(eaW[:, DK:2 * DK], AW[:])
            eab_row = const.tile([1, 2 * DK], f32r, tag="eab_row")
            nc.sync.dma_start(eab_row[0:1, 0:DK], EB[:].rearrange("(o k) -> o k", o=1))
            nc.sync.dma_start(eab_row[0:1, DK:2 * DK], AB_[:].rearrange("(o k) -> o k", o=1))
            ones_row = const.tile([1, DK], f32r, tag="ones_row")
            nc.vector.memset(ones_row[:].bitcast(fp32), 1.0)
            onesCH = const.tile([1, CH * 128], f32r, tag="onesCH")
            nc.vector.memset(onesCH[:].bitcast(fp32), 1.0)
            patCH2 = const.tile([1, CH * 512], f32r, tag="patCH2")
            nc.vector.memset(patCH2[:].bitcast(fp32), 0.0)
            nc.vector.memset(
                patCH2[:].rearrange("p (t x) -> p t x", x=256)[:, :, 0:DK].bitcast(fp32), 1.0)

            fW1 = const.tile([DK, DK], f32r, tag="fW1")
            nc.sync.dma_start(fW1[:], FW[0:DK, :])
            fW2 = const.tile([DK, DK], f32r, tag="fW2")
            nc.sync.dma_start(fW2[:], FW[DK:2 * DK, :])
            fb_col = const.tile([DK, 1], fp32, tag="fb_col")
            nc.sync.dma_start(fb_col[:], FB[:].rearrange("(k o) -> k o", o=1))
            pW = const.tile([DK, 1], f32r, tag="pW")
            nc.sync.dma_start(pW[:], PW[:])
            pb_t = const.tile([1, 1], fp32, tag="pb_t")
            nc.sync.dma_start(pb_t[:], PB[:].rearrange("(o k) -> o k", o=1))

            mv0_t = const.tile([128, DK], f32r, tag="mv0_t")
            nc.sync.dma_start(mv0_t[0:DV, :], MV0[:])
            nc.sync.dma_start(mv0_t[DV:128, :], MV0[:])

            psK_cm.__exit__(None, None, None)
            for rep in range(nrep):
                psA_cm = tc.tile_pool(name="psA", bufs=3, space="PSUM")
                psA = psA_cm.__enter__()
                # ---------------- indices ----------------
                qidx = const.tile([128, 16], i32, tag="qidx")
                cidx = const.tile([128, 16], i32, tag="cidx")
                vidx = const.tile([128, 16], i32, tag="vidx")
                # row r = 128c + p ; p = tl*8 + b ; t = 16c + tl
                qsrc = QID[:].rearrange("(c tl) (b two) -> tl b c two", tl=16, two=2)[:, :, :, 0]
                nc.sync.dma_start(qidx[:, 0:NTILE], qsrc)
                csrc = COR[:].rearrange("(c tl) (b two) -> tl b c two", tl=16, two=2)[:, :, :, 0]
                nc.sync.dma_start(cidx[:, 0:NTILE], csrc)
                nc.vector.scalar_tensor_tensor(
                    out=vidx[:, 0:NTILE], in0=cidx[:, 0:NTILE], scalar=NQ,
                    in1=qidx[:, 0:NTILE], op0=AL.mult, op1=AL.add)

                # ---------------- gather + transpose ----------------
                kT = bigp.tile([DK, ROWS], f32r, tag="kT")
                vT = bigp.tile([DK, ROWS], f32r, tag="vT")
                for c in range(NTILE):
                    sl = slice(128 * c, 128 * c + 128)
                    k_t = work.tile([128, DK], fp32, tag="gath")
                    nc.gpsimd.indirect_dma_start(
                        out=k_t[:], out_offset=None, in_=KEMB[:],
                        in_offset=bass.IndirectOffsetOnAxis(ap=qidx[:, c:c + 1], axis=0))
                    kt_ps = psA.tile([128, 512], fp32, space="PSUM", tag="tps")
                    nc.tensor.transpose(kt_ps[:, 0:128], k_t[:], ident[:])
                    nc.vector.tensor_copy(kT[:, sl], kt_ps[:, 0:128])

                    v_t = work.tile([128, DK], fp32, tag="gath")
                    nc.gpsimd.indirect_dma_start(
                        out=v_t[:], out_offset=None, in_=VEMB[:],
                        in_offset=bass.IndirectOffsetOnAxis(ap=vidx[:, c:c + 1], axis=0))
                    vt_ps = psA.tile([128, 512], fp32, space="PSUM", tag="tps")
                    nc.tensor.transpose(vt_ps[:, 0:128], v_t[:], ident[:])
                    nc.vector.tensor_copy(vT[:, sl], vt_ps[:, 0:128])

                # ---------------- gates: w softmax, e, a ----------------
                wT2 = bigp.tile([128, ROWS], fp32, tag="wT2")   # rows 0-63: wT ; rows 64-127: copy
                for c in range(NTILE):
                    sl = slice(128 * c, 128 * c + 128)
                    lps = psA.tile([128, 512], fp32, space="PSUM", tag="tps")
                    nc.tensor.matmul(lps[:, 0:DV], lhsT=kT[:, sl], rhs=mkT[:],
                                     start=True, stop=True)
                    negmax = small.tile([128, 1], fp32, tag="nm")
                    nc.vector.tensor_reduce(negmax[:], lps[:, 0:DV], AX.X, AL.max,
                                            negate=True)
                    exp_sb = work.tile([128, DV], fp32, tag="exp")
                    sumexp = small.tile([128, 1], fp32, tag="se")
                    nc.scalar.activation(exp_sb[:], lps[:, 0:DV], AF.Exp,
                                         bias=negmax[:, 0:1], accum_out=sumexp[:, 0:1])
                    rec = small.tile([128, 1], fp32, tag="rec")
                    nc.vector.reciprocal(rec[:], sumexp[:])
                    w_sb = work.tile([128, DV], fp32, tag="wsb")
                    nc.vector.tensor_scalar_mul(w_sb[:], exp_sb[:], rec[:, 0:1])
                    nc.sync.dma_start(W_D[128 * c:128 * c + 128, :], R(w_sb[:]))
                    # transpose w for the read-selector
                    wt_ps = psA.tile([128, 512], fp32, space="PSUM", tag="tps")
                    nc.tensor.transpose(wt_ps[0:DV, 0:128], w_sb[:], ident[:])
                    nc.vector.tensor_copy(wT2[0:DV, sl], wt_ps[0:DV, 0:128])

                    eps_full = psA.tile([128, 512], fp32, space="PSUM", tag="tps")
                    eps = eps_full[:, 0:2 * DK]
                    nc.tensor.matmul(eps, lhsT=vT[:, sl], rhs=eaW[:],
                                     start=True, stop=False)
                    nc.tensor.matmul(eps, lhsT=ones_row[:], rhs=eab_row[:],
                                     start=False, stop=True)
                    ea_sb = work.tile([128, 2 * DK], fp32, tag="easb")
                    nc.scalar.activation(ea_sb[:, 0:DK], eps_full[:, 0:DK], AF.Sigmoid)
                    nc.scalar.activation(ea_sb[:, DK:2 * DK], eps_full[:, DK:2 * DK], AF.Tanh)
                    # negate e half (store [-e | a])
                    nc.vector.tensor_scalar_mul(ea_sb[:, 0:DK], ea_sb[:, 0:DK], -1.0)
                    nc.sync.dma_start(EA_D[128 * c:128 * c + 128, :], R(ea_sb[:]))

                # replicate wT to partitions 64-127 (SBUF->SBUF partition shift)
                nc.sync.dma_start(wT2[DV:128, :], wT2[0:DV, :])

                # read-selector: WcvZ[(c,v), (t, g, c')] = w_t[2g+c, v] if c'==c else 0
                wcvz = scanp.tile([128, S * BL], f32r, tag="wcvz")
                nc.vector.memset(wcvz[:].bitcast(fp32), 0.0)
                wv_u = wT2[0:DV, :].rearrange("p (t b) -> p t b", b=BL)
                wz_u = wcvz[0:DV, :].rearrange("p (t g c) -> p t g c", g=4, c=2)
                nc.vector.tensor_copy(wz_u[:, :, :, 0], wv_u[:, 0:S, 0::2])
                wv_l = wT2[DV:128, :].rearrange("p (t b) -> p t b", b=BL)
                wz_l = wcvz[DV:128, :].rearrange("p (t g c) -> p t g c", g=4, c=2)
                nc.vector.tensor_copy(wz_l[:, :, :, 1], wv_l[:, 0:S, 1::2])

                psA_cm.__exit__(None, None, None)
                psS_cm = tc.tile_pool(name="psS", bufs=2, space="PSUM")
                psS = psS_cm.__enter__()
                psRp_cm = tc.tile_pool(name="psRp", bufs=2, space="PSUM")
                psRp = psRp_cm.__enter__()

                # ---------------- Mv init ----------------
                mv_cur = mvp.tile([128, 4 * DK], f32r, tag="mv")
                for g in range(4):
                    nc.vector.tensor_copy(mv_cur[:, DK * g:DK * g + DK], mv0_t[:])

                # ---------------- scan buffers ----------------
                w9b = []
                ea9b = []
                for i in range(2):
                    t9 = scanp.tile([9, CH * 128], f32r, tag=f"w9_{i}")
                    nc.vector.memset(t9[0:8, :].bitcast(fp32), 0.0)
                    nc.sync.dma_start(t9[8:9, :], onesCH[:])
                    w9b.append(t9)
                    gl = []
                    for j in range(2):
                        te = scanp.tile([9, CH * 512], f32r, tag=f"ea9_{i}_{j}")
                        nc.vector.memset(te[0:8, :].bitcast(fp32), 0.0)
                        nc.sync.dma_start(te[8:9, :], patCH2[:])
                        gl.append(te)
                    ea9b.append(gl)

                # zero the pad region of reads scratch
                zpad = const.tile([64, DK], fp32, tag="zpad")
                nc.vector.memset(zpad[:], 0.0)
                nc.sync.dma_start(R_D[S * BL:ROWS, :], zpad[:])

                # ---------------- the scan ----------------
                rc = None
                for ch in range(NCH):
                    buf = ch % 2
                    t0 = ch * CH
                    for b in range(BL) if "refill" not in abl else []:
                        h = b % 2
                        dst_w = w9b[buf][b:b + 1, :].rearrange(
                            "p (t x) -> p t x", x=128)[:, :, 64 * h:64 * h + 64]
                        src_w = W_D[:].rearrange("(t b) v -> b t v", b=BL)[b, t0:t0 + CH, :]
                        nc.sync.dma_start(dst_w, src_w.rearrange("(o t) v -> o t v", o=1))
                        j, h2 = b // 4, (b // 2) % 2
                        dst_e = ea9b[buf][j][b:b + 1, :].rearrange(
                            "p (t x) -> p t x", x=512)[:, :, 256 * h2:256 * h2 + 256]
                        src_e = EA_D[:].rearrange("(t b) k -> b t k", b=BL)[b, t0:t0 + CH, :]
                        nc.sync.dma_start(dst_e, src_e.rearrange("(o t) k -> o t k", o=1))

                    for tl in range(CH):
                        t = t0 + tl
                        psab = psS.tile([128, 1024], fp32, space="PSUM", tag="psab")
                        if "abmm" not in abl:
                            for j in range(2):
                                nc.tensor.matmul(
                                    psab[:, 512 * j:512 * j + 512],
                                    lhsT=w9b[buf][0:9, 128 * tl:128 * tl + 128],
                                    rhs=ea9b[buf][j][0:9, 512 * tl:512 * tl + 512],
                                    start=True, stop=True)
                        psr = psRp.tile([8, 512], fp32, space="PSUM", tag="psr")
                        if "readmm" not in abl:
                            nc.tensor.matmul(
                                psr[:], lhsT=wcvz[:, 8 * t:8 * t + 8],
                                rhs=mv_cur[:], start=True, stop=True)
                        if "dve" not in abl:
                            psab_v = psab[:].rearrange("p (g x) -> p g x", g=4)
                            tT = ttp.tile([128, 4 * DK], fp32, tag="tt")
                            nc.vector.tensor_tensor(
                                out=tT[:].rearrange("p (g x) -> p g x", g=4),
                                in0=mv_cur[:].rearrange("p (g x) -> p g x", g=4),
                                in1=psab_v[:, :, 0:DK], op=AL.mult)
                            mv_next = mvp.tile([128, 4 * DK], f32r, tag="mv")
                            nc.vector.tensor_tensor(
                                out=mv_next[:].rearrange("p (g x) -> p g x", g=4),
                                in0=tT[:].rearrange("p (g x) -> p g x", g=4),
                                in1=psab_v[:, :, DK:2 * DK], op=AL.add)
                            mv_cur = mv_next
                        # reads drain
                        if "drain" in abl:
                            continue
                        if t % 4 == 0:
                            rc = rcp.tile([8, 4 * 512], fp32, tag="rc")
                        nc.scalar.copy(rc[:, 512 * (t % 4):512 * (t % 4) + 512], psr[:])
                        if t % 4 == 3 and "rcdma" not in abl:
                            for g in range(4):
                                dstg = R_D[:].rearrange(
                                    "(t b) k -> t b k", b=BL)[t - 3:t + 1, 2 * g:2 * g + 2, :]
                                srcg = rc[2 * g:2 * g + 2, :].rearrange(
                                    "c (t x) -> c t x", x=512)[:, :, 128 * g:128 * g + 128]
                                nc.sync.dma_start(
                                    dstg.rearrange("t c k -> c t k"), srcg)

                psRp_cm.__exit__(None, None, None)
                psS_cm.__exit__(None, None, None)
                psC_cm = tc.tile_pool(name="psC", bufs=3, space="PSUM")
                psC = psC_cm.__enter__()

                # ---------------- phase C ----------------
                readsT = bigp.tile([DK, ROWS], f32r, tag="readsT")
                for c in range(NTILE):
                    sl = slice(128 * c, 128 * c + 128)
                    r_t = work.tile([128, DK], fp32, tag="gath")
                    nc.sync.dma_start(r_t[:], R_D[128 * c:128 * c + 128, :])
                    rt_ps = psC.tile([128, 512], fp32, space="PSUM", tag="cps")
                    nc.tensor.transpose(rt_ps[:, 0:128], r_t[:], ident[:])
                    nc.vector.tensor_copy(readsT[:, sl], rt_ps[:, 0:128])

                fT = bigp.tile([DK, ROWS], f32r, tag="fT")
                out_sb = const.tile([1, ROWS], fp32, tag="out_sb")
                for c0 in range(0, ROWS, 512):
                    w_ = min(512, ROWS - c0)
                    sl = slice(c0, c0 + w_)
                    fps = psC.tile([128, 512], fp32, space="PSUM", tag="cps")
                    nc.tensor.matmul(fps[:, 0:w_], lhsT=fW1[:], rhs=readsT[:, sl],
                                     start=True, stop=False)
                    nc.tensor.matmul(fps[:, 0:w_], lhsT=fW2[:], rhs=kT[:, sl],
                                     start=False, stop=True)
                    nc.scalar.activation(fT[:, sl], fps[:, 0:w_], AF.Tanh,
                                         bias=fb_col[:, 0:1])
                    pps = psC.tile([2, 512], fp32, space="PSUM", tag="cpr")
                    nc.tensor.matmul(pps[0:1, 0:w_], lhsT=pW[:], rhs=fT[:, sl],
                                     start=True, stop=True)
                    nc.scalar.activation(out_sb[0:1, sl], pps[0:1, 0:w_], AF.Sigmoid,
                                         bias=pb_t[0:1, 0:1])
                nc.sync.dma_start(OUT[:], out_sb[:])
                psC_cm.__exit__(None, None, None)

    nc.finalize()
    return nc


def make_in_maps(inputs):
    def prep_idx(a):
        # [BL, S] int -> t-major padded little-endian int32 view [TP, 2*BL]
        a = np.ascontiguousarray(np.asarray(a).astype(np.int64, copy=False).T)  # [S, BL]
        v = a.view(np.int32).reshape(S, 2 * BL)
        out = np.zeros((TP, 2 * BL), np.int32)
        out[:S] = v
        return out

    common = {
        "key_emb": np.ascontiguousarray(inputs["key_emb"], np.float32),
        "value_emb": np.ascontiguousarray(inputs["value_emb"], np.float32),
        "Mk": np.ascontiguousarray(inputs["Mk"], np.float32),
        "Mv0": np.ascontiguousarray(inputs["Mv0"], np.float32),
        "f_W": np.ascontiguousarray(inputs["f_W"], np.float32),
        "f_b": np.ascontiguousarray(inputs["f_b"], np.float32),
        "e_W": np.ascontiguousarray(inputs["e_W"], np.float32),
        "e_b": np.ascontiguousarray(inputs["e_b"], np.float32),
        "a_W": np.ascontiguousarray(inputs["a_W"], np.float32),
        "a_b": np.ascontiguousarray(inputs["a_b"], np.float32),
        "p_W": np.ascontiguousarray(inputs["p_W"], np.float32),
        "p_b": np.ascontiguousarray(inputs["p_b"], np.float32),
    }
    in_maps = []
    for core in range(NCORES):
        bs = slice(core * BL, core * BL + BL)
        m = dict(common)
        m["qid32"] = prep_idx(np.asarray(inputs["question_seq"])[bs])
        m["cor32"] = prep_idx(np.asarray(inputs["correctness_seq"])[bs])
        in_maps.append(m)
    return in_maps


def kernel(**inputs):
    from concourse.bass_utils import run_bass_kernel_spmd

    if "nc" not in _CACHE:
        _CACHE["nc"] = _build()
    nc = _CACHE["nc"]
    in_maps = make_in_maps(inputs)
    _CACHE["in_maps"] = in_maps
    res = run_bass_kernel_spmd(nc, in_maps, core_ids=list(range(NCORES)))
    out = np.empty((B, S), np.float32)
    for core in range(NCORES):
        flat = res.results[core]["out"].reshape(ROWS)
        out[core * BL:(core + 1) * BL, :] = flat[:S * BL].reshape(S, BL).T
    return out



# revision 43
# speedup vs baseline: 2.4942x; 1.3551x over previous
"""DKVMN forward kernel for Trainium2 (8 NeuronCores, batch-parallel).

Strategy (per core, 8 batches):
  Phase A: indirect-DMA gather of key/value embedding rows (device-side
    index math), PE transposes, PE matmuls for softmax logits / erase /
    add gates, softmax via DVE+ACT.
  Scan (t = 0..199, fully unrolled): per step one PE-stationary load of a
    [9,128] block-selector (8 batches' attention weights placed in a
    (c,v) selector layout + a ones row) and 4 matmuls against per-group
    zero-padded [-e | a] moving operands produce PSUM [1 - w*e | w*a]
    directly.  DVE then needs only 2 tensor_tensor passes:
       T = Mv * A ;  Mv' = T + B.
    Reads r_t = w_t . Mv_{t-1} are 4 tiny PE matmuls with [128,2]
    stationaries, drained by ACT and DMA'd to DRAM scratch.
  Phase C: reload reads, PE transpose, two accumulated matmuls with f_W,
    tanh (+bias), p_W matvec, sigmoid -> [1, 1664] output per core.

Host side only slices/pads/reinterprets inputs and reassembles outputs.
"""

import numpy as np

B, S, DK, DV, NQ = 64, 200, 128, 64, 10000
NCORES = 8
BL = B // NCORES          # 8 batches per core
TP = 208                  # t padded to 13*128/8
NTILE = 13                # (t,b)-row tiles of 128 -> 1664 rows
ROWS = TP * BL            # 1664
CH = 10                   # scan chunk length (steps)
NCH = S // CH             # 8 chunks

_CACHE = {}


def _build(nrep=1, abl=()):
    import concourse.bacc as bacc
    import concourse.bass as bass
    import concourse.mybir as mybir
    from concourse.tile import TileContext
    from concourse.masks import make_identity

    fp32 = mybir.dt.float32
    f32r = mybir.dt.float32r
    i32 = mybir.dt.int32

    def R(ap):
        return ap.bitcast(f32r)
    AL = mybir.AluOpType
    AF = mybir.ActivationFunctionType
    AX = mybir.AxisListType

    nc = bacc.Bacc(None)

    QID = nc.dram_tensor("qid32", [TP, 2 * BL], i32, kind="ExternalInput")
    COR = nc.dram_tensor("cor32", [TP, 2 * BL], i32, kind="ExternalInput")
    KEMB = nc.dram_tensor("key_emb", [NQ, DK], fp32, kind="ExternalInput")
    VEMB = nc.dram_tensor("value_emb", [2 * NQ, DK], fp32, kind="ExternalInput")
    MK = nc.dram_tensor("Mk", [DV, DK], fp32, kind="ExternalInput")
    MV0 = nc.dram_tensor("Mv0", [DV, DK], f32r, kind="ExternalInput")
    FW = nc.dram_tensor("f_W", [2 * DK, DK], f32r, kind="ExternalInput")
    FB = nc.dram_tensor("f_b", [DK], fp32, kind="ExternalInput")
    EW = nc.dram_tensor("e_W", [DK, DK], f32r, kind="ExternalInput")
    EB = nc.dram_tensor("e_b", [DK], f32r, kind="ExternalInput")
    AW = nc.dram_tensor("a_W", [DK, DK], f32r, kind="ExternalInput")
    AB_ = nc.dram_tensor("a_b", [DK], f32r, kind="ExternalInput")
    PW = nc.dram_tensor("p_W", [DK, 1], f32r, kind="ExternalInput")
    PB = nc.dram_tensor("p_b", [1], fp32, kind="ExternalInput")

    W_D = nc.dram_tensor("w_scratch", [ROWS, DV], f32r, kind="Internal")
    R_D = nc.dram_tensor("reads_scratch", [ROWS, DK], fp32, kind="Internal")
    EA_D = nc.dram_tensor("ea_scratch", [ROWS, 2 * DK], f32r, kind="Internal")
    OUT = nc.dram_tensor("out", [1, ROWS], fp32, kind="ExternalOutput")

    with TileContext(nc) as tc:
        with tc.tile_pool(name="const", bufs=1) as const, \
             tc.tile_pool(name="big", bufs=1) as bigp, \
             tc.tile_pool(name="scan", bufs=1) as scanp, \
             tc.tile_pool(name="work", bufs=3) as work, \
             tc.tile_pool(name="small", bufs=4) as small, \
             tc.tile_pool(name="mv", bufs=2) as mvp, \
             tc.tile_pool(name="tt", bufs=2) as ttp, \
             tc.tile_pool(name="rc", bufs=2) as rcp:

            psK_cm = tc.tile_pool(name="psK", bufs=2, space="PSUM")
            psK = psK_cm.__enter__()

            # ---------------- constants ----------------
            ident = const.tile([128, 128], fp32, tag="ident")
            make_identity(nc, ident[:])

            mk_sb = const.tile([DV, DK], fp32, tag="mk_sb")
            nc.sync.dma_start(mk_sb[:], MK[:])
            mkT_ps = psK.tile([128, 512], fp32, space="PSUM", tag="kps")
            nc.tensor.transpose(mkT_ps[0:DK, 0:DV], mk_sb[:], ident[0:DV, 0:DV])
            mkT = const.tile([DK, DV], f32r, tag="mkT")
            nc.vector.tensor_copy(mkT[:], mkT_ps[0:DK, 0:DV])

            eaW = const.tile([DK, 2 * DK], f32r, tag="eaW")
            nc.sync.dma_start(eaW[:, 0:DK], EW[:])
            nc.sync.dma_start(eaW[:, DK:2 * DK], AW[:])
            eab_row = const.tile([1, 2 * DK], f32r, tag="eab_row")
            nc.sync.dma_start(eab_row[0:1, 0:DK], EB[:].rearrange("(o k) -> o k", o=1))
            nc.sync.dma_start(eab_row[0:1, DK:2 * DK], AB_[:].rearrange("(o k) -> o k", o=1))
            ones_row = const.tile([1, DK], f32r, tag="ones_row")
            nc.vector.memset(ones_row[:].bitcast(fp32), 1.0)
            onesCH = const.tile([1, CH * 128], f32r, tag="onesCH")
            nc.vector.memset(onesCH[:].bitcast(fp32), 1.0)

            fW1 = const.tile([DK, DK], f32r, tag="fW1")
            nc.sync.dma_start(fW1[:], FW[0:DK, :])
            fW2 = const.tile([DK, DK], f32r, tag="fW2")
            nc.sync.dma_start(fW2[:], FW[DK:2 * DK, :])
            fb_col = const.tile([DK, 1], fp32, tag="fb_col")
            nc.sync.dma_start(fb_col[:], FB[:].rearrange("(k o) -> k o", o=1))
            pW = const.tile([DK, 1], f32r, tag="pW")
            nc.sync.dma_start(pW[:], PW[:])
            pb_t = const.tile([1, 1], fp32, tag="pb_t")
            nc.sync.dma_start(pb_t[:], PB[:].rearrange("(o k) -> o k", o=1))

            mv0_t = const.tile([128, DK], f32r, tag="mv0_t")
            nc.sync.dma_start(mv0_t[0:DV, :], MV0[:])
            nc.sync.dma_start(mv0_t[DV:128, :], MV0[:])

            psK_cm.__exit__(None, None, None)
            for rep in range(nrep):
                psA_cm = tc.tile_pool(name="psA", bufs=3, space="PSUM")
                psA = psA_cm.__enter__()
                # ---------------- indices ----------------
                qidx = const.tile([128, 16], i32, tag="qidx")
                cidx = const.tile([128, 16], i32, tag="cidx")
                vidx = const.tile([128, 16], i32, tag="vidx")
                # row r = 128c + p ; p = tl*8 + b ; t = 16c + tl
                qsrc = QID[:].rearrange("(c tl) (b two) -> tl b c two", tl=16, two=2)[:, :, :, 0]
                nc.sync.dma_start(qidx[:, 0:NTILE], qsrc)
                csrc = COR[:].rearrange("(c tl) (b two) -> tl b c two", tl=16, two=2)[:, :, :, 0]
                nc.sync.dma_start(cidx[:, 0:NTILE], csrc)
                nc.vector.scalar_tensor_tensor(
                    out=vidx[:, 0:NTILE], in0=cidx[:, 0:NTILE], scalar=NQ,
                    in1=qidx[:, 0:NTILE], op0=AL.mult, op1=AL.add)

                # ---------------- gather + transpose ----------------
                kT = bigp.tile([DK, ROWS], f32r, tag="kT")
                vT = bigp.tile([DK, ROWS], f32r, tag="vT")
                for c in range(NTILE):
                    sl = slice(128 * c, 128 * c + 128)
                    k_t = work.tile([128, DK], fp32, tag="gath")
                    nc.gpsimd.indirect_dma_start(
                        out=k_t[:], out_offset=None, in_=KEMB[:],
                        in_offset=bass.IndirectOffsetOnAxis(ap=qidx[:, c:c + 1], axis=0))
                    kt_ps = psA.tile([128, 512], fp32, space="PSUM", tag="tps")
                    nc.tensor.transpose(kt_ps[:, 0:128], k_t[:], ident[:])
                    nc.vector.tensor_copy(kT[:, sl], kt_ps[:, 0:128])

                    v_t = work.tile([128, DK], fp32, tag="gath")
                    nc.gpsimd.indirect_dma_start(
                        out=v_t[:], out_offset=None, in_=VEMB[:],
                        in_offset=bass.IndirectOffsetOnAxis(ap=vidx[:, c:c + 1], axis=0))
                    vt_ps = psA.tile([128, 512], fp32, space="PSUM", tag="tps")
                    nc.tensor.transpose(vt_ps[:, 0:128], v_t[:], ident[:])
                    nc.vector.tensor_copy(vT[:, sl], vt_ps[:, 0:128])

                # ---------------- gates: w softmax, e, a ----------------
                wT2 = bigp.tile([128, ROWS], fp32, tag="wT2")   # rows 0-63: wT ; rows 64-127: copy
                for c in range(NTILE):
                    sl = slice(128 * c, 128 * c + 128)
                    lps = psA.tile([128, 512], fp32, space="PSUM", tag="tps")
                    nc.tensor.matmul(lps[:, 0:DV], lhsT=kT[:, sl], rhs=mkT[:],
                                     start=True, stop=True)
                    negmax = small.tile([128, 1], fp32, tag="nm")
                    nc.vector.tensor_reduce(negmax[:], lps[:, 0:DV], AX.X, AL.max,
                                            negate=True)
                    exp_sb = work.tile([128, DV], fp32, tag="exp")
                    sumexp = small.tile([128, 1], fp32, tag="se")
                    nc.scalar.activation(exp_sb[:], lps[:, 0:DV], AF.Exp,
                                         bias=negmax[:, 0:1], accum_out=sumexp[:, 0:1])
                    rec = small.tile([128, 1], fp32, tag="rec")
                    nc.vector.reciprocal(rec[:], sumexp[:])
                    w_sb = work.tile([128, DV], fp32, tag="wsb")
                    nc.vector.tensor_scalar_mul(w_sb[:], exp_sb[:], rec[:, 0:1])
                    nc.sync.dma_start(W_D[128 * c:128 * c + 128, :], R(w_sb[:]))
                    # transpose w for the read-selector
                    wt_ps = psA.tile([128, 512], fp32, space="PSUM", tag="tps")
                    nc.tensor.transpose(wt_ps[0:DV, 0:128], w_sb[:], ident[:])
                    nc.vector.tensor_copy(wT2[0:DV, sl], wt_ps[0:DV, 0:128])

                    eps_full = psA.tile([128, 512], fp32, space="PSUM", tag="tps")
                    eps = eps_full[:, 0:2 * DK]
                    nc.tensor.matmul(eps, lhsT=vT[:, sl], rhs=eaW[:],
                                     start=True, stop=False)
                    nc.tensor.matmul(eps, lhsT=ones_row[:], rhs=eab_row[:],
                                     start=False, stop=True)
                    ea_sb = work.tile([128, 2 * DK], fp32, tag="easb")
                    nc.scalar.activation(ea_sb[:, 0:DK], eps_full[:, 0:DK], AF.Sigmoid)
                    nc.scalar.activation(ea_sb[:, DK:2 * DK], eps_full[:, DK:2 * DK], AF.Tanh)
                    # negate e half (store [-e | a])
                    nc.vector.tensor_scalar_mul(ea_sb[:, 0:DK], ea_sb[:, 0:DK], -1.0)
                    nc.sync.dma_start(EA_D[128 * c:128 * c + 128, :], R(ea_sb[:]))

                # replicate wT to partitions 64-127 (SBUF->SBUF partition shift)
                nc.sync.dma_start(wT2[DV:128, :], wT2[0:DV, :])

                # read-selector: WcvZ[(c,v), (t, g, c')] = w_t[2g+c, v] if c'==c else 0
                wcvz = scanp.tile([128, S * BL], f32r, tag="wcvz")
                nc.vector.memset(wcvz[:].bitcast(fp32), 0.0)
                wv_u = wT2[0:DV, :].rearrange("p (t b) -> p t b", b=BL)
                wz_u = wcvz[0:DV, :].rearrange("p (t g c) -> p t g c", g=4, c=2)
                nc.vector.tensor_copy(wz_u[:, :, :, 0], wv_u[:, 0:S, 0::2])
                wv_l = wT2[DV:128, :].rearrange("p (t b) -> p t b", b=BL)
                wz_l = wcvz[DV:128, :].rearrange("p (t g c) -> p t g c", g=4, c=2)
                nc.vector.tensor_copy(wz_l[:, :, :, 1], wv_l[:, 0:S, 1::2])

                psA_cm.__exit__(None, None, None)
                psS_cm = tc.tile_pool(name="psS", bufs=2, space="PSUM")
                psS = psS_cm.__enter__()
                psRp_cm = tc.tile_pool(name="psRp", bufs=2, space="PSUM")
                psRp = psRp_cm.__enter__()

                # ---------------- Mv init ----------------
                mv_cur = mvp.tile([128, 4 * DK], f32r, tag="mv")
                for g in range(4):
                    nc.vector.tensor_copy(mv_cur[:, DK * g:DK * g + DK], mv0_t[:])

                # ---------------- scan buffers ----------------
                w9b = []
                ea9b = []
                for i in range(2):
                    t9 = scanp.tile([9, CH * 128], f32r, tag=f"w9_{i}")
                    nc.vector.memset(t9[0:9, :].bitcast(fp32), 0.0)
                    nc.sync.dma_start(t9[8:9, :], onesCH[:])
                    w9b.append(t9)
                    gl = []
                    for j in range(2):
                        te = scanp.tile([9, CH * 512], f32r, tag=f"ea9_{i}_{j}")
                        nc.vector.memset(te[0:9, :].bitcast(fp32), 0.0)
                        tev = te[8:9, :].rearrange("p (t x) -> p t x", x=512)
                        ocv = onesCH[:].rearrange("p (t x) -> p t x", x=DK)
                        nc.sync.dma_start(tev[:, :, 0:DK], ocv)
                        nc.sync.dma_start(tev[:, :, 256:256 + DK], ocv)
                        gl.append(te)
                    ea9b.append(gl)

                rs_wide = bigp.tile([128, NTILE * 512], fp32, tag="rs_wide")
                nc.vector.memset(rs_wide[:], 0.0)
                stg = None

                # ---------------- the scan ----------------
                rc = None
                for ch in range(NCH):
                    buf = ch % 2
                    t0 = ch * CH
                    for b in range(BL) if "refill" not in abl else []:
                        h = b % 2
                        dst_w = w9b[buf][b:b + 1, :].rearrange(
                            "p (t x) -> p t x", x=128)[:, :, 64 * h:64 * h + 64]
                        src_w = W_D[:].rearrange("(t b) v -> b t v", b=BL)[b, t0:t0 + CH, :]
                        nc.sync.dma_start(dst_w, src_w.rearrange("(o t) v -> o t v", o=1))
                        j, h2 = b // 4, (b // 2) % 2
                        dst_e = ea9b[buf][j][b:b + 1, :].rearrange(
                            "p (t x) -> p t x", x=512)[:, :, 256 * h2:256 * h2 + 256]
                        src_e = EA_D[:].rearrange("(t b) k -> b t k", b=BL)[b, t0:t0 + CH, :]
                        nc.sync.dma_start(dst_e, src_e.rearrange("(o t) k -> o t k", o=1))

                    for tl in range(CH):
                        t = t0 + tl
                        psab = psS.tile([128, 1024], fp32, space="PSUM", tag="psab")
                        if "abmm" not in abl:
                            for j in range(2):
                                nc.tensor.matmul(
                                    psab[:, 512 * j:512 * j + 512],
                                    lhsT=w9b[buf][0:9, 128 * tl:128 * tl + 128],
                                    rhs=ea9b[buf][j][0:9, 512 * tl:512 * tl + 512],
                                    start=True, stop=True)
                        psr = psRp.tile([8, 512], fp32, space="PSUM", tag="psr")
                        if "readmm" not in abl:
                            nc.tensor.matmul(
                                psr[:], lhsT=wcvz[:, 8 * t:8 * t + 8],
                                rhs=mv_cur[:], start=True, stop=True)
                        if "dve" not in abl:
                            psab_v = psab[:].rearrange("p (g x) -> p g x", g=4)
                            tT = ttp.tile([128, 4 * DK], fp32, tag="tt")
                            nc.vector.tensor_tensor(
                                out=tT[:].rearrange("p (g x) -> p g x", g=4),
                                in0=mv_cur[:].rearrange("p (g x) -> p g x", g=4),
                                in1=psab_v[:, :, 0:DK], op=AL.mult)
                            mv_next = mvp.tile([128, 4 * DK], f32r, tag="mv")
                            nc.vector.tensor_tensor(
                                out=mv_next[:].rearrange("p (g x) -> p g x", g=4),
                                in0=tT[:].rearrange("p (g x) -> p g x", g=4),
                                in1=psab_v[:, :, DK:2 * DK], op=AL.add)
                            mv_cur = mv_next
                        # reads drain via stg + per-8-step scatter DMA
                        if "drain" in abl:
                            continue
                        m, tau, t8 = t // 16, t % 16, t % 8
                        if t8 == 0:
                            stg = rcp.tile([8, 8 * 512], fp32, tag="rc")
                        nc.scalar.copy(
                            stg[0:8, 512 * t8:512 * t8 + 512], psr[:])
                        if t8 == 7 or t == S - 1:
                            nt = t8 + 1
                            z = tau - t8
                            rwv = rs_wide[:].rearrange(
                                "(t b) X -> t b X", b=BL)
                            for b in range(BL):
                                nc.gpsimd.dma_start(
                                    rwv[z:z + nt, b, 512 * m:512 * m + 512],
                                    stg[b:b + 1, 0:512 * nt].rearrange(
                                        "p (t x) -> p t x", x=512))

                psRp_cm.__exit__(None, None, None)
                psS_cm.__exit__(None, None, None)
                psC_cm = tc.tile_pool(name="psC", bufs=3, space="PSUM")
                psC = psC_cm.__enter__()

                # ---------------- phase C ----------------
                readsT = bigp.tile([DK, ROWS], f32r, tag="readsT")
                rtv = readsT[:].rearrange("p (m t b) -> p m t b", t=16, b=BL)
                for c in range(NTILE):
                    for g in range(4):
                        rt_ps = psC.tile([128, 512], fp32, space="PSUM", tag="cps")
                        nc.tensor.transpose(
                            rt_ps[0:128, 0:128],
                            rs_wide[:, 512 * c + 128 * g:512 * c + 128 * g + 128],
                            ident[:])
                        tv = rt_ps[0:128, 0:128].rearrange(
                            "p (t b) -> p t b", b=BL)
                        nc.vector.tensor_copy(
                            rtv[:, c, :, 2 * g:2 * g + 2], tv[:, :, 2 * g:2 * g + 2])

                fT = vT
                osb = const.tile([1, 512], fp32, tag="osb")
                for c0 in range(0, ROWS, 512):
                    w_ = min(512, ROWS - c0)
                    sl = slice(c0, c0 + w_)
                    fps = psC.tile([128, 512], fp32, space="PSUM", tag="cps")
                    nc.tensor.matmul(fps[:, 0:w_], lhsT=fW1[:], rhs=readsT[:, sl],
                                     start=True, stop=False)
                    nc.tensor.matmul(fps[:, 0:w_], lhsT=fW2[:], rhs=kT[:, sl],
                                     start=False, stop=True)
                    nc.scalar.activation(fT[:, sl], fps[:, 0:w_], AF.Tanh,
                                         bias=fb_col[:, 0:1])
                    pps = psC.tile([2, 512], fp32, space="PSUM", tag="cpr")
                    nc.tensor.matmul(pps[0:1, 0:w_], lhsT=pW[:], rhs=fT[:, sl],
                                     start=True, stop=True)
                    nc.scalar.activation(osb[0:1, 0:w_], pps[0:1, 0:w_], AF.Sigmoid,
                                         bias=pb_t[0:1, 0:1])
                    nc.sync.dma_start(OUT[0:1, sl], osb[0:1, 0:w_])
                psC_cm.__exit__(None, None, None)

    nc.finalize()
    return nc


def make_in_maps(inputs):
    def prep_idx(a):
        # [BL, S] int -> t-major padded little-endian int32 view [TP, 2*BL]
        a = np.ascontiguousarray(np.asarray(a).astype(np.int64, copy=False).T)  # [S, BL]
        v = a.view(np.int32).reshape(S, 2 * BL)
        out = np.zeros((TP, 2 * BL), np.int32)
        out[:S] = v
        return out

    common = {
        "key_emb": np.ascontiguousarray(inputs["key_emb"], np.float32),
        "value_emb": np.ascontiguousarray(inputs["value_emb"], np.float32),
        "Mk": np.ascontiguousarray(inputs["Mk"], np.float32),
        "Mv0": np.ascontiguousarray(inputs["Mv0"], np.float32),
        "f_W": np.ascontiguousarray(inputs["f_W"], np.float32),
        "f_b": np.ascontiguousarray(inputs["f_b"], np.float32),
        "e_W": np.ascontiguousarray(inputs["e_W"], np.float32),
        "e_b": np.ascontiguousarray(inputs["e_b"], np.float32),
        "a_W": np.ascontiguousarray(inputs["a_W"], np.float32),
        "a_b": np.ascontiguousarray(inputs["a_b"], np.float32),
        "p_W": np.ascontiguousarray(inputs["p_W"], np.float32),
        "p_b": np.ascontiguousarray(inputs["p_b"], np.float32),
    }
    in_maps = []
    for core in range(NCORES):
        bs = slice(core * BL, core * BL + BL)
        m = dict(common)
        m["qid32"] = prep_idx(np.asarray(inputs["question_seq"])[bs])
        m["cor32"] = prep_idx(np.asarray(inputs["correctness_seq"])[bs])
        in_maps.append(m)
    return in_maps


def kernel(**inputs):
    from concourse.bass_utils import run_bass_kernel_spmd

    if "nc" not in _CACHE:
        _CACHE["nc"] = _build()
    nc = _CACHE["nc"]
    in_maps = make_in_maps(inputs)
    _CACHE["in_maps"] = in_maps
    res = run_bass_kernel_spmd(nc, in_maps, core_ids=list(range(NCORES)))
    out = np.empty((B, S), np.float32)
    for core in range(NCORES):
        flat = res.results[core]["out"].reshape(ROWS)
        out[core * BL:(core + 1) * BL, :] = flat[:S * BL].reshape(S, BL).T
    return out

